# revision 20
# baseline (speedup 1.0000x reference)
"""Trainium2 Bass kernel for nn_CausalDiscoveryModule (topk_masking), v3.

Math (per batch b):
    c = MLP(context_vec[b]); u = c*c  (E=32)
    S[i,j] = sum_e u_e * emb[i,e] * emb[j,e]          (N=512 x N=512 logits)
    adj = sigmoid(S); keep top-10 per row (mask), times gate = sigmoid(Linear(c))

v3 vs v1: ranking runs on sig = sigmoid(S) directly (monotone, so same
order as S; S in [-12, 17] so sig is strictly inside (0,1) with no fp32
saturation plateaus -- verified 2/131072 rows carry a sigma-fp32 tie at
the top-10/11 boundary, the same fp32-ambiguity class the S-domain
ranking already has).  This removes the PSUM->SBUF eviction pass: the
ACT engine only does Sigmoid (PSUM read) + gate-scale, and the DVE does
its irreducible 4 passes (max8, match_replace, max8, masked-mult) which
are the kernel's bottleneck (2376 ns/tile).

Device algorithm per [128 x 512] tile:
    PE   : S = A_b.T @ E_T (K=32 contraction, fp32)   A_b = E_T scaled by u_b
    ACT  : sig = Sigmoid(S_psum); sig_g = sig * gate_b
    DVE  : m1 = top8(sig); Z1 = match_replace(sig, m1, -BIG)
           m2 = top8(Z1) -> v9,v10 at slots 0,1
           out = (sig >= v10) * sig_g
Sharding: data-parallel over batch, 32 batches per core.
"""

import os
import sys

import numpy as np

try:
    import concourse.bass as bass  # noqa: F401
except ImportError:
    sys.path.insert(0, "/opt/trn_rl_repo")

import concourse.bacc as bacc
import concourse.mybir as mybir
from concourse import tile
from concourse.bass_utils import run_bass_kernel_spmd

F32 = mybir.dt.float32
AF = mybir.ActivationFunctionType
ALU = mybir.AluOpType

B, N, E, IN_DIM, TOP_K = 256, 512, 32, 512, 10
N_CORES = 8
BL = B // N_CORES          # 32 batches per core
IBLK = N // 128            # 4 i-blocks per batch
NEG_BIG = -1.0e30

_COMPILED = None
LAST_RESULTS = None


def _build():
    nc = bacc.Bacc(
        "TRN2",
        target_bir_lowering=False,
        debug=False,
        num_devices=N_CORES,
    )

    # DRAM I/O (host marshals layouts; shapes below are per-core)
    ctxT_d = nc.dram_tensor("ctxT", [IN_DIM, BL], F32, kind="ExternalInput").ap()
    embT_d = nc.dram_tensor("embT", [E, N], F32, kind="ExternalInput").ap()
    w1T_d = nc.dram_tensor("w1T", [IN_DIM, E], F32, kind="ExternalInput").ap()
    b1_d = nc.dram_tensor("b1", [E, 1], F32, kind="ExternalInput").ap()
    w2T_d = nc.dram_tensor("w2T", [E, E], F32, kind="ExternalInput").ap()
    b2_d = nc.dram_tensor("b2", [E, 1], F32, kind="ExternalInput").ap()
    wgT_d = nc.dram_tensor("wgT", [E, 1], F32, kind="ExternalInput").ap()
    bg_d = nc.dram_tensor("bg", [1, 1], F32, kind="ExternalInput").ap()
    ones_d = nc.dram_tensor("ones1", [1, 128], F32, kind="ExternalInput").ap()
    out_d = nc.dram_tensor("out", [BL * N, N], F32, kind="ExternalOutput").ap()
    out_v = out_d.rearrange("(b k p) j -> b k p j", b=BL, k=IBLK, p=128)

    with tile.TileContext(nc) as tc:
        with (
            tc.tile_pool(name="const", bufs=1) as cpool,
            tc.tile_pool(name="apool", bufs=2) as apool,
            tc.tile_pool(name="sigp", bufs=4) as sigp,
            tc.tile_pool(name="sgp", bufs=4) as sgp,
            tc.tile_pool(name="z1p", bufs=4) as z1p,
            tc.tile_pool(name="outp", bufs=4) as outp,
            tc.tile_pool(name="m8p", bufs=8) as m8p,
            tc.tile_pool(name="psum", bufs=6, space="PSUM") as pp,
            tc.tile_pool(name="psmall", bufs=2, space="PSUM") as pps,
        ):
            # ---- warm the ACT tables first ----
            # The framework inserts LoadActFuncSet (1283ns) before the first
            # use of each table; dummy activations on a framework const tile
            # (memset at t=0, no wait deps) pull both loads to the very start
            # of the ACT stream, overlapping the weight DMAs.
            dummy = nc.const_aps.aps[(F32, 0.0)][0:1, 0:1]
            dummy_o = cpool.tile([1, 4], F32)
            nc.scalar.activation(dummy_o[:, 0:1], dummy, AF.Sigmoid)
            nc.scalar.activation(dummy_o[:, 1:2], dummy, AF.Relu)
            nc.scalar.activation(dummy_o[:, 2:3], dummy, AF.Identity)
            nc.scalar.activation(dummy_o[:, 3:4], dummy, AF.Square)

            # ---- load constants / weights ----
            # Spread preamble DMAs across 3 queues (each DMA serializes on
            # the shared HWDGE ~500ns) in critical-path order.
            ctxT = cpool.tile([128, 4, BL], F32)    # context.T in 4 chunks of 128
            w1T = cpool.tile([128, 4, E], F32)
            embT = cpool.tile([E, N], F32)          # var_emb.T
            b1s = cpool.tile([E, 1], F32)
            w2T = cpool.tile([E, E], F32)
            b2s = cpool.tile([E, 1], F32)
            wgT = cpool.tile([E, 1], F32)
            bgs = cpool.tile([1, 1], F32)
            ones1 = cpool.tile([1, 128], F32)
            loads = [
                (nc.gpsimd, w1T, w1T_d.rearrange("(c p) e -> p c e", c=4)),
                (nc.sync, ctxT, ctxT_d.rearrange("(c p) l -> p c l", c=4)),
                (nc.scalar, b1s, b1_d),
                (nc.gpsimd, w2T, w2T_d),
                (nc.sync, embT, embT_d),
                (nc.scalar, b2s, b2_d),
                (nc.gpsimd, wgT, wgT_d),
                (nc.scalar, bgs, bg_d),
                (nc.sync, ones1, ones_d),
            ]
            for q, dst, src in loads:
                q.dma_start(out=dst, in_=src)

            # ---- context MLP (transposed): hT = relu(W1T.T @ ctxT + b1) ----
            hT_ps = pps.tile([E, BL], F32, tag="mlp")
            for c in range(4):
                nc.tensor.matmul(
                    out=hT_ps, lhsT=w1T[:, c, :], rhs=ctxT[:, c, :],
                    start=(c == 0), stop=(c == 3),
                )
            hT = cpool.tile([E, BL], F32)
            nc.scalar.activation(hT, hT_ps, AF.Relu, bias=b1s[:, 0:1])

            cT_ps = pps.tile([E, BL], F32, tag="mlp")
            nc.tensor.matmul(out=cT_ps, lhsT=w2T, rhs=hT, start=True, stop=True)
            # u = c*c fused from PSUM (Square(x + b2)); cT is only needed by
            # the gate chain, so its eviction is deferred there too
            uT = cpool.tile([E, BL], F32)
            nc.scalar.activation(uT, cT_ps, AF.Square, bias=b2s[:, 0:1])
            cT = cpool.tile([E, BL], F32)

            # gate tiles; the gate chain itself is emitted after the first
            # tile's matmul+sigmoid so it stays off the ramp critical path
            # (in-order engines: emitting it earlier would delay A_t / S / sig)
            g1 = cpool.tile([1, BL], F32)
            g_all = cpool.tile([128, BL], F32)

            # ---- main loop ----
            for b in range(BL):
                # A_t = embT * u_b on ACT (ACT has slack; DVE is the bottleneck)
                A_t = apool.tile([E, N], F32)
                if b == 0:
                    # chunked so the first matmul starts after 1/2 of A_t
                    for c in range(2):
                        nc.scalar.activation(
                            A_t[:, c * 256:(c + 1) * 256],
                            embT[:, c * 256:(c + 1) * 256],
                            AF.Identity, scale=uT[:, b:b + 1],
                        )
                else:
                    nc.scalar.activation(A_t, embT, AF.Identity, scale=uT[:, b:b + 1])
                for k in range(IBLK):
                    first = b == 0 and k == 0
                    S_ps = pp.tile([128, N], F32)
                    sig = sigp.tile([128, N], F32)
                    nc.tensor.matmul(
                        out=S_ps, lhsT=A_t[:, k * 128:(k + 1) * 128], rhs=embT,
                        start=True, stop=True,
                    )
                    # tile 0 ranks on S straight from PSUM; its DVE ops are
                    # emitted BEFORE sigma so the scheduler doesn't chain the
                    # first max8 behind sigma's completion (same-tile readers
                    # get serialized by emission order).
                    rank_in = S_ps if first else sig
                    m1 = m8p.tile([128, 8], F32, tag="m1")
                    Z1 = z1p.tile([128, N], F32)
                    m2 = m8p.tile([128, 8], F32, tag="m2")
                    if first:
                        nc.vector.max(out=m1, in_=rank_in)
                        nc.vector.match_replace(
                            out=Z1, in_to_replace=m1, in_values=rank_in,
                            imm_value=NEG_BIG,
                        )
                        nc.vector.max(out=m2, in_=Z1)
                    nc.scalar.activation(sig, S_ps, AF.Sigmoid)
                    if first:
                        # gate: g = sigmoid(Wg @ c + bg) broadcast to 128 rows
                        nc.scalar.activation(cT, cT_ps, AF.Identity, bias=b2s[:, 0:1])
                        g_ps = pps.tile([1, BL], F32, tag="mlp")
                        nc.tensor.matmul(out=g_ps, lhsT=wgT, rhs=cT, start=True, stop=True)
                        nc.scalar.activation(g1, g_ps, AF.Sigmoid, bias=bgs[:, 0:1])
                        ga_ps = pps.tile([128, BL], F32, tag="mlp")
                        nc.tensor.matmul(out=ga_ps, lhsT=ones1, rhs=g1, start=True, stop=True)
                        nc.scalar.activation(g_all, ga_ps, AF.Identity)
                    sig_g = None
                    if not first:
                        sig_g = sgp.tile([128, N], F32)
                        nc.scalar.mul(sig_g, sig, g_all[:, b:b + 1])

                    if not first:
                        nc.vector.max(out=m1, in_=rank_in)
                        nc.vector.match_replace(
                            out=Z1, in_to_replace=m1, in_values=rank_in,
                            imm_value=NEG_BIG,
                        )
                        nc.vector.max(out=m2, in_=Z1)
                    # mask = (sig >= v10): exact top-10 for rows whose v10 is
                    # bitwise-unique in fp32 sigma space (2/131072 rows tie;
                    # same ambiguity class the harness envelope covers).
                    out_t = outp.tile([128, N], F32)
                    if first:
                        # mask against sig (ready right after sigma), apply
                        # the gate afterward via a 2x tensor_scalar: avoids
                        # the 4th DVE op stalling ~800ns on the gate chain
                        tmp = sgp.tile([128, N], F32)
                        nc.vector.scalar_tensor_tensor(
                            out=tmp, in0=rank_in, scalar=m2[:, 1:2], in1=sig,
                            op0=ALU.is_ge, op1=ALU.mult,
                        )
                        nc.vector.tensor_scalar(
                            out=out_t, in0=tmp, scalar1=g_all[:, b:b + 1],
                            scalar2=None, op0=ALU.mult,
                        )
                        nc.sync.dma_start(out=out_v[b, k], in_=out_t)
                    elif b == BL - 1 and k == IBLK - 1:
                        # last tile: mask+DMA in shrinking chunks, one per
                        # DMA-capable queue (no issue serialization), so the
                        # final DMA after the last DVE op only moves 32KB
                        dq = [nc.gpsimd, nc.sync, nc.scalar]
                        bounds = [0, 256, 448, 512]
                        for q in range(3):
                            sl = slice(bounds[q], bounds[q + 1])
                            nc.vector.scalar_tensor_tensor(
                                out=out_t[:, sl], in0=sig[:, sl],
                                scalar=m2[:, 1:2], in1=sig_g[:, sl],
                                op0=ALU.is_ge, op1=ALU.mult,
                            )
                            dq[q].dma_start(out=out_v[b, k][:, sl], in_=out_t[:, sl])
                    else:
                        nc.vector.scalar_tensor_tensor(
                            out=out_t, in0=rank_in, scalar=m2[:, 1:2], in1=sig_g,
                            op0=ALU.is_ge, op1=ALU.mult,
                        )
                        nc.sync.dma_start(out=out_v[b, k], in_=out_t)

    nc.compile()
    return nc


def _get_compiled():
    global _COMPILED
    if _COMPILED is None:
        _COMPILED = _build()
    return _COMPILED


def _make_in_maps(inputs):
    cv = np.asarray(inputs["context_vec"], dtype=np.float32)
    emb = np.asarray(inputs["var_emb"], dtype=np.float32)
    W1 = np.asarray(inputs["W1"], dtype=np.float32)
    b1 = np.asarray(inputs["b1"], dtype=np.float32)
    W2 = np.asarray(inputs["W2"], dtype=np.float32)
    b2 = np.asarray(inputs["b2"], dtype=np.float32)
    Wg = np.asarray(inputs["Wg"], dtype=np.float32)
    bg = np.asarray(inputs["bg"], dtype=np.float32)

    shared = {
        "embT": np.ascontiguousarray(emb.T),
        "w1T": np.ascontiguousarray(W1.T),
        "b1": np.ascontiguousarray(b1.reshape(E, 1)),
        "w2T": np.ascontiguousarray(W2.T),
        "b2": np.ascontiguousarray(b2.reshape(E, 1)),
        "wgT": np.ascontiguousarray(Wg.reshape(1, E).T),
        "bg": np.ascontiguousarray(bg.reshape(1, 1)),
        "ones1": np.ones((1, 128), dtype=np.float32),
    }
    in_maps = []
    for k in range(N_CORES):
        m = dict(shared)
        m["ctxT"] = np.ascontiguousarray(cv[k * BL:(k + 1) * BL, :].T)
        in_maps.append(m)
    return in_maps


def kernel(**inputs) -> np.ndarray:
    global LAST_RESULTS
    nc = _get_compiled()
    in_maps = _make_in_maps(inputs)
    trace = os.environ.get("BASS_KERNEL_TRACE", "0") == "1"
    try:
        res = run_bass_kernel_spmd(
            nc, in_maps, core_ids=list(range(N_CORES)), trace=trace,
        )
    except ModuleNotFoundError:
        res = run_bass_kernel_spmd(
            nc, in_maps, core_ids=list(range(N_CORES)), trace=False,
        )
    LAST_RESULTS = res
    parts = [np.asarray(r["out"]).reshape(BL, N, N) for r in res.results]
    return np.concatenate(parts, axis=0)


# revision 23
# speedup vs baseline: 1.0009x; 1.0009x over previous
"""Trainium2 Bass kernel for nn_CausalDiscoveryModule (topk_masking), v3.

Math (per batch b):
    c = MLP(context_vec[b]); u = c*c  (E=32)
    S[i,j] = sum_e u_e * emb[i,e] * emb[j,e]          (N=512 x N=512 logits)
    adj = sigmoid(S); keep top-10 per row (mask), times gate = sigmoid(Linear(c))

v3 vs v1: ranking runs on sig = sigmoid(S) directly (monotone, so same
order as S; S in [-12, 17] so sig is strictly inside (0,1) with no fp32
saturation plateaus -- verified 2/131072 rows carry a sigma-fp32 tie at
the top-10/11 boundary, the same fp32-ambiguity class the S-domain
ranking already has).  This removes the PSUM->SBUF eviction pass: the
ACT engine only does Sigmoid (PSUM read) + gate-scale, and the DVE does
its irreducible 4 passes (max8, match_replace, max8, masked-mult) which
are the kernel's bottleneck (2376 ns/tile).

Device algorithm per [128 x 512] tile:
    PE   : S = A_b.T @ E_T (K=32 contraction, fp32)   A_b = E_T scaled by u_b
    ACT  : sig = Sigmoid(S_psum); sig_g = sig * gate_b
    DVE  : m1 = top8(sig); Z1 = match_replace(sig, m1, -BIG)
           m2 = top8(Z1) -> v9,v10 at slots 0,1
           out = (sig >= v10) * sig_g
Sharding: data-parallel over batch, 32 batches per core.
"""

import os
import sys

import numpy as np

try:
    import concourse.bass as bass  # noqa: F401
except ImportError:
    sys.path.insert(0, "/opt/trn_rl_repo")

import concourse.bacc as bacc
import concourse.mybir as mybir
from concourse import tile
from concourse.bass_utils import run_bass_kernel_spmd

F32 = mybir.dt.float32
AF = mybir.ActivationFunctionType
ALU = mybir.AluOpType

B, N, E, IN_DIM, TOP_K = 256, 512, 32, 512, 10
N_CORES = 8
BL = B // N_CORES          # 32 batches per core
IBLK = N // 128            # 4 i-blocks per batch
NEG_BIG = -1.0e30

_COMPILED = None
LAST_RESULTS = None


def _build():
    nc = bacc.Bacc(
        "TRN2",
        target_bir_lowering=False,
        debug=False,
        num_devices=N_CORES,
    )

    # DRAM I/O (host marshals layouts; shapes below are per-core)
    ctxT_d = nc.dram_tensor("ctxT", [IN_DIM, BL], F32, kind="ExternalInput").ap()
    embT_d = nc.dram_tensor("embT", [E, N], F32, kind="ExternalInput").ap()
    w1T_d = nc.dram_tensor("w1T", [IN_DIM, E], F32, kind="ExternalInput").ap()
    b1_d = nc.dram_tensor("b1", [E, 1], F32, kind="ExternalInput").ap()
    w2T_d = nc.dram_tensor("w2T", [E, E], F32, kind="ExternalInput").ap()
    b2_d = nc.dram_tensor("b2", [E, 1], F32, kind="ExternalInput").ap()
    wgT_d = nc.dram_tensor("wgT", [E, 1], F32, kind="ExternalInput").ap()
    bg_d = nc.dram_tensor("bg", [1, 1], F32, kind="ExternalInput").ap()
    ones_d = nc.dram_tensor("ones1", [1, 128], F32, kind="ExternalInput").ap()
    out_d = nc.dram_tensor("out", [BL * N, N], F32, kind="ExternalOutput").ap()
    out_v = out_d.rearrange("(b k p) j -> b k p j", b=BL, k=IBLK, p=128)

    with tile.TileContext(nc) as tc:
        with (
            tc.tile_pool(name="const", bufs=1) as cpool,
            tc.tile_pool(name="apool", bufs=2) as apool,
            tc.tile_pool(name="sigp", bufs=4) as sigp,
            tc.tile_pool(name="sgp", bufs=4) as sgp,
            tc.tile_pool(name="z1p", bufs=4) as z1p,
            tc.tile_pool(name="outp", bufs=4) as outp,
            tc.tile_pool(name="m8p", bufs=8) as m8p,
            tc.tile_pool(name="psum", bufs=6, space="PSUM") as pp,
            tc.tile_pool(name="psmall", bufs=2, space="PSUM") as pps,
        ):
            # ---- warm the ACT tables first ----
            # The framework inserts LoadActFuncSet (1283ns) before the first
            # use of each table; dummy activations on a framework const tile
            # (memset at t=0, no wait deps) pull both loads to the very start
            # of the ACT stream, overlapping the weight DMAs.
            dummy = nc.const_aps.aps[(F32, 0.0)][0:1, 0:1]
            dummy_o = cpool.tile([1, 4], F32)
            nc.scalar.activation(dummy_o[:, 0:1], dummy, AF.Sigmoid)
            nc.scalar.activation(dummy_o[:, 1:2], dummy, AF.Relu)
            nc.scalar.activation(dummy_o[:, 2:3], dummy, AF.Identity)
            nc.scalar.activation(dummy_o[:, 3:4], dummy, AF.Square)

            # ---- load constants / weights ----
            # Spread preamble DMAs across 3 queues (each DMA serializes on
            # the shared HWDGE ~500ns) in critical-path order.
            ctxT = cpool.tile([128, 4, BL], F32)    # context.T in 4 chunks of 128
            w1T = cpool.tile([128, 4, E], F32)
            embT = cpool.tile([E, N], F32)          # var_emb.T
            b1s = cpool.tile([E, 1], F32)
            w2T = cpool.tile([E, E], F32)
            b2s = cpool.tile([E, 1], F32)
            wgT = cpool.tile([E, 1], F32)
            bgs = cpool.tile([1, 1], F32)
            ones1 = cpool.tile([1, 128], F32)
            loads = [
                (nc.gpsimd, w1T, w1T_d.rearrange("(c p) e -> p c e", c=4)),
                (nc.sync, ctxT, ctxT_d.rearrange("(c p) l -> p c l", c=4)),
                (nc.scalar, b1s, b1_d),
                (nc.gpsimd, w2T, w2T_d),
                (nc.sync, embT, embT_d),
                (nc.scalar, b2s, b2_d),
                (nc.gpsimd, wgT, wgT_d),
                (nc.scalar, bgs, bg_d),
                (nc.sync, ones1, ones_d),
            ]
            for q, dst, src in loads:
                q.dma_start(out=dst, in_=src)

            # ---- context MLP (transposed): hT = relu(W1T.T @ ctxT + b1) ----
            hT_ps = pps.tile([E, BL], F32, tag="mlp")
            for c in range(4):
                nc.tensor.matmul(
                    out=hT_ps, lhsT=w1T[:, c, :], rhs=ctxT[:, c, :],
                    start=(c == 0), stop=(c == 3),
                )
            hT = cpool.tile([E, BL], F32)
            nc.scalar.activation(hT, hT_ps, AF.Relu, bias=b1s[:, 0:1])

            cT_ps = pps.tile([E, BL], F32, tag="mlp")
            nc.tensor.matmul(out=cT_ps, lhsT=w2T, rhs=hT, start=True, stop=True)
            # u = c*c fused from PSUM (Square(x + b2)); cT is only needed by
            # the gate chain, so its eviction is deferred there too
            uT = cpool.tile([E, BL], F32)
            nc.scalar.activation(uT, cT_ps, AF.Square, bias=b2s[:, 0:1])
            cT = cpool.tile([E, BL], F32)

            # gate tiles; the gate chain itself is emitted after the first
            # tile's matmul+sigmoid so it stays off the ramp critical path
            # (in-order engines: emitting it earlier would delay A_t / S / sig)
            g1 = cpool.tile([1, BL], F32)
            g_all = cpool.tile([128, BL], F32)

            # ---- main loop ----
            for b in range(BL):
                # A_t = embT * u_b on ACT (ACT has slack; DVE is the bottleneck)
                A_t = apool.tile([E, N], F32)
                if b == 0:
                    # k=0 only needs A_t cols 0:128 -- emit that chunk first
                    # so the first matmul starts as early as possible
                    nc.scalar.activation(
                        A_t[:, 0:128], embT[:, 0:128],
                        AF.Identity, scale=uT[:, b:b + 1],
                    )
                    nc.scalar.activation(
                        A_t[:, 128:N], embT[:, 128:N],
                        AF.Identity, scale=uT[:, b:b + 1],
                    )
                else:
                    nc.scalar.activation(A_t, embT, AF.Identity, scale=uT[:, b:b + 1])
                for k in range(IBLK):
                    first = b == 0 and k == 0
                    S_ps = pp.tile([128, N], F32)
                    sig = sigp.tile([128, N], F32)
                    nc.tensor.matmul(
                        out=S_ps, lhsT=A_t[:, k * 128:(k + 1) * 128], rhs=embT,
                        start=True, stop=True,
                    )
                    # tile 0 ranks on S straight from PSUM; its DVE ops are
                    # emitted BEFORE sigma so the scheduler doesn't chain the
                    # first max8 behind sigma's completion (same-tile readers
                    # get serialized by emission order).
                    rank_in = S_ps if first else sig
                    m1 = m8p.tile([128, 8], F32, tag="m1")
                    Z1 = z1p.tile([128, N], F32)
                    m2 = m8p.tile([128, 8], F32, tag="m2")
                    if first:
                        nc.vector.max(out=m1, in_=rank_in)
                        nc.vector.match_replace(
                            out=Z1, in_to_replace=m1, in_values=rank_in,
                            imm_value=NEG_BIG,
                        )
                        nc.vector.max(out=m2, in_=Z1)
                    nc.scalar.activation(sig, S_ps, AF.Sigmoid)
                    if first:
                        # gate: g = sigmoid(Wg @ c + bg) broadcast to 128 rows
                        nc.scalar.activation(cT, cT_ps, AF.Identity, bias=b2s[:, 0:1])
                        g_ps = pps.tile([1, BL], F32, tag="mlp")
                        nc.tensor.matmul(out=g_ps, lhsT=wgT, rhs=cT, start=True, stop=True)
                        nc.scalar.activation(g1, g_ps, AF.Sigmoid, bias=bgs[:, 0:1])
                        ga_ps = pps.tile([128, BL], F32, tag="mlp")
                        nc.tensor.matmul(out=ga_ps, lhsT=ones1, rhs=g1, start=True, stop=True)
                        nc.scalar.activation(g_all, ga_ps, AF.Identity)
                    sig_g = None
                    if not first:
                        sig_g = sgp.tile([128, N], F32)
                        nc.scalar.mul(sig_g, sig, g_all[:, b:b + 1])

                    if not first:
                        nc.vector.max(out=m1, in_=rank_in)
                        nc.vector.match_replace(
                            out=Z1, in_to_replace=m1, in_values=rank_in,
                            imm_value=NEG_BIG,
                        )
                        nc.vector.max(out=m2, in_=Z1)
                    # mask = (sig >= v10): exact top-10 for rows whose v10 is
                    # bitwise-unique in fp32 sigma space (2/131072 rows tie;
                    # same ambiguity class the harness envelope covers).
                    out_t = outp.tile([128, N], F32)
                    if first:
                        # mask against sig (ready right after sigma), apply
                        # the gate afterward via a 2x tensor_scalar: avoids
                        # the 4th DVE op stalling ~800ns on the gate chain
                        tmp = sgp.tile([128, N], F32)
                        nc.vector.scalar_tensor_tensor(
                            out=tmp, in0=rank_in, scalar=m2[:, 1:2], in1=sig,
                            op0=ALU.is_ge, op1=ALU.mult,
                        )
                        nc.vector.tensor_scalar(
                            out=out_t, in0=tmp, scalar1=g_all[:, b:b + 1],
                            scalar2=None, op0=ALU.mult,
                        )
                        nc.sync.dma_start(out=out_v[b, k], in_=out_t)
                    elif b == BL - 1 and k == IBLK - 1:
                        # last tile: asymmetric [384|128] mask+DMA on separate
                        # queues -- same total DVE cycles as halves, but the
                        # final DMA after the last DVE op only moves 512B rows
                        dq = [nc.gpsimd, nc.sync]
                        bounds = [0, 384, 512]
                        for q in range(2):
                            sl = slice(bounds[q], bounds[q + 1])
                            nc.vector.scalar_tensor_tensor(
                                out=out_t[:, sl], in0=sig[:, sl],
                                scalar=m2[:, 1:2], in1=sig_g[:, sl],
                                op0=ALU.is_ge, op1=ALU.mult,
                            )
                            dq[q].dma_start(out=out_v[b, k][:, sl], in_=out_t[:, sl])
                    else:
                        nc.vector.scalar_tensor_tensor(
                            out=out_t, in0=rank_in, scalar=m2[:, 1:2], in1=sig_g,
                            op0=ALU.is_ge, op1=ALU.mult,
                        )
                        nc.sync.dma_start(out=out_v[b, k], in_=out_t)

    nc.compile()
    return nc


def _get_compiled():
    global _COMPILED
    if _COMPILED is None:
        _COMPILED = _build()
    return _COMPILED


def _make_in_maps(inputs):
    cv = np.asarray(inputs["context_vec"], dtype=np.float32)
    emb = np.asarray(inputs["var_emb"], dtype=np.float32)
    W1 = np.asarray(inputs["W1"], dtype=np.float32)
    b1 = np.asarray(inputs["b1"], dtype=np.float32)
    W2 = np.asarray(inputs["W2"], dtype=np.float32)
    b2 = np.asarray(inputs["b2"], dtype=np.float32)
    Wg = np.asarray(inputs["Wg"], dtype=np.float32)
    bg = np.asarray(inputs["bg"], dtype=np.float32)

    shared = {
        "embT": np.ascontiguousarray(emb.T),
        "w1T": np.ascontiguousarray(W1.T),
        "b1": np.ascontiguousarray(b1.reshape(E, 1)),
        "w2T": np.ascontiguousarray(W2.T),
        "b2": np.ascontiguousarray(b2.reshape(E, 1)),
        "wgT": np.ascontiguousarray(Wg.reshape(1, E).T),
        "bg": np.ascontiguousarray(bg.reshape(1, 1)),
        "ones1": np.ones((1, 128), dtype=np.float32),
    }
    in_maps = []
    for k in range(N_CORES):
        m = dict(shared)
        m["ctxT"] = np.ascontiguousarray(cv[k * BL:(k + 1) * BL, :].T)
        in_maps.append(m)
    return in_maps


def kernel(**inputs) -> np.ndarray:
    global LAST_RESULTS
    nc = _get_compiled()
    in_maps = _make_in_maps(inputs)
    trace = os.environ.get("BASS_KERNEL_TRACE", "0") == "1"
    try:
        res = run_bass_kernel_spmd(
            nc, in_maps, core_ids=list(range(N_CORES)), trace=trace,
        )
    except ModuleNotFoundError:
        res = run_bass_kernel_spmd(
            nc, in_maps, core_ids=list(range(N_CORES)), trace=False,
        )
    LAST_RESULTS = res
    parts = [np.asarray(r["out"]).reshape(BL, N, N) for r in res.results]
    return np.concatenate(parts, axis=0)
